# revision 9
# baseline (speedup 1.0000x reference)
"""Causal self-attention (B=4, T=2048, C=1024, H=16) on 8 TRN2 NeuronCores.

Sharding: data-parallel over batch (4) x tensor-parallel over head-halves (2).
Core g handles batch g//2 and heads [8*(g%2), 8*(g%2)+8). Megatron-style:
Wq/Wk/Wv column-sharded, Wp row-sharded; the host sums the two partial y
contributions per batch and adds the (bv @ Wp + bp) term (valid because
softmax rows sum to 1, so the v-bias passes through attention).

v2 design (vs the f32r baseline):
  - all matmul operands bf16 (inputs converted host-side): halves DMA/SBUF,
    enables FWL weight loads, no narrow-N f32r penalty on straddle tiles.
  - x^T fully resident in SBUF; projections re-read it from SBUF instead of
    re-streaming 8MB from HBM per head-pair.
  - stage pipeline: for s in 0..3: project(tq=s) -> attention(j=s) -> y(j=s).
    Causality makes attention j=s depend only on projections tq<=s, so the
    PE stays dense and the exp stream starts ~20us into the kernel.
  - S-pair matmuls (two heads row-packed at partitions 0/64) write one
    2-bank PSUM tile [128,1024]; ONE ScalarE exp covers both heads, halving
    ACT instruction count.
  - normalize without gpsimd: reciprocal of the sums rows straight from
    PSUM (lane 64), broadcast to 64 partitions via K=1 PE matmuls, then DVE
    multiplies. Odd head still needs one small SBUF->SBUF DMA shift.
  - y projection accumulates in PSUM and DMAs straight to HBM (no copy).
"""

import math

import numpy as np
import ml_dtypes

import concourse.bass as bass
import concourse.tile as tile
from concourse import bacc, mybir
from concourse.bass_utils import run_bass_kernel_spmd

B, T, C, H = 4, 2048, 1024, 16
D = C // H  # 64
N_CORES = 8
F = C // 2  # 512 features per core (8 heads)
FT = F // 128  # 4 feature tiles (head pairs) per core
CCH = C // 128  # 8 contraction chunks
NQ = T // 512  # 4 q-tiles / stages
NKT = T // 128  # 16 k-tiles
SCALE = 1.0 / math.sqrt(D)

f32 = mybir.dt.float32
f32r = mybir.dt.float32r
bf16 = mybir.dt.bfloat16

_cache = {}
DEBUG_DUMPS = False


def _build():
    nc = bacc.Bacc("TRN2", target_bir_lowering=False, debug=False,
                   num_devices=N_CORES)
    xT = nc.dram_tensor("xT", [C, T], bf16, kind="ExternalInput").ap()
    wq = nc.dram_tensor("wq", [FT, 128, CCH, 128], bf16, kind="ExternalInput").ap()
    wk = nc.dram_tensor("wk", [FT, 128, CCH, 128], bf16, kind="ExternalInput").ap()
    wv = nc.dram_tensor("wv", [C, F], bf16, kind="ExternalInput").ap()
    wp = nc.dram_tensor("wp", [F, C], bf16, kind="ExternalInput").ap()
    bqk = nc.dram_tensor("bqk", [2, F], f32, kind="ExternalInput").ap()
    cmask = nc.dram_tensor("cmask", [128, 264], bf16, kind="ExternalInput").ap()
    y = nc.dram_tensor("y", [T, C], f32, kind="ExternalOutput").ap()
    dbg = None
    if DEBUG_DUMPS:
        dbg = {
            "d_qT0": nc.dram_tensor("d_qT0", [128, T], bf16,
                                    kind="ExternalOutput").ap(),
            "d_kT0": nc.dram_tensor("d_kT0", [128, T], bf16,
                                    kind="ExternalOutput").ap(),
            "d_v0": nc.dram_tensor("d_v0", [128, H // 2, D + 1], bf16,
                                   kind="ExternalOutput").ap(),
            "d_pt00": nc.dram_tensor("d_pt00", [128, 1024], bf16,
                                     kind="ExternalOutput").ap(),
            "d_oT0": nc.dram_tensor("d_oT0", [128, T], bf16,
                                    kind="ExternalOutput").ap(),
            "d_oT1": nc.dram_tensor("d_oT1", [128, T], bf16,
                                    kind="ExternalOutput").ap(),
            "d_oT2": nc.dram_tensor("d_oT2", [128, T], bf16,
                                    kind="ExternalOutput").ap(),
            "d_oT3": nc.dram_tensor("d_oT3", [128, T], bf16,
                                    kind="ExternalOutput").ap(),
            "d_recs0": nc.dram_tensor("d_recs0", [1, 1024], f32,
                                      kind="ExternalOutput").ap(),
        }

    with tile.TileContext(nc) as tc:
        _body(tc, xT, wq, wk, wv, wp, bqk, cmask, y, dbg)
    nc.compile()
    return nc


def _body(tc, xT, wq, wk, wv, wp, bqk, cmask, y, dbg=None):
    nc = tc.nc
    Exp = mybir.ActivationFunctionType.Exp

    pools = []

    def pool(**kw):
        p = tc.alloc_tile_pool(**kw)
        pools.append(p)
        return p

    consts = pool(name="consts", bufs=1)
    big = pool(name="big", bufs=1)
    qkt_pool = pool(name="qkt", bufs=1)
    v_pool = pool(name="v", bufs=1)
    ot_pool = pool(name="ot", bufs=1)
    pt_pool = pool(name="pt", bufs=8)
    norm_pool = pool(name="norm", bufs=2)
    tmp_pool = pool(name="tmp", bufs=2)
    ps_misc = pool(name="ps_misc", bufs=2, space="PSUM")
    ps_s = pool(name="ps_s", bufs=2, space="PSUM")
    ps_o = pool(name="ps_o", bufs=2, space="PSUM")

    xT_r = xT.rearrange("(k p) t -> p k t", p=128)

    # ---- resident inputs, DMA'd in the order the first stage needs ----
    cmask_sb = consts.tile([128, 264], bf16, tag="cmask")
    nc.sync.dma_start(out=cmask_sb[:], in_=cmask[:])
    bqk_sb = consts.tile([128, 2, FT], f32, tag="bqk")
    nc.sync.dma_start(out=bqk_sb[:], in_=bqk.rearrange("b (f p) -> p b f", p=128))
    # Per-queue DMA bandwidth is ~1/16 of the core's total, so the startup
    # transfers are split into per-chunk dma_starts that spread round-robin
    # across queues; otherwise the first projection waits ~20us on one queue.
    x_sb = big.tile([128, CCH, T], bf16, tag="x")
    for cc in range(CCH):
        nc.sync.dma_start(out=x_sb[:, cc, 0:512], in_=xT_r[:, cc, 0:512])
    wq_sb = big.tile([128, FT, CCH, 128], bf16, tag="wq")
    wk_sb = big.tile([128, FT, CCH, 128], bf16, tag="wk")
    wq_r = wq.rearrange("f p k c -> p f k c")
    wk_r = wk.rearrange("f p k c -> p f k c")
    for half in range(2):
        cs = slice(half * 4, half * 4 + 4)
        nc.sync.dma_start(out=wq_sb[:, 0, cs], in_=wq_r[:, 0, cs])
        nc.sync.dma_start(out=wk_sb[:, 0, cs], in_=wk_r[:, 0, cs])
    wv_sb = big.tile([128, CCH, F], bf16, tag="wv")
    wv_r = wv.rearrange("(k p) f -> p k f", p=128)
    for cc in range(0, CCH, 2):
        nc.sync.dma_start(out=wv_sb[:, cc:cc + 2], in_=wv_r[:, cc:cc + 2])
    for hp in range(1, FT):
        nc.sync.dma_start(out=wq_sb[:, hp], in_=wq_r[:, hp])
        nc.sync.dma_start(out=wk_sb[:, hp], in_=wk_r[:, hp])
    for tq in range(1, NQ):
        ts = slice(tq * 512, (tq + 1) * 512)
        for cc in range(0, CCH, 2):
            nc.sync.dma_start(out=x_sb[:, cc:cc + 2, ts],
                              in_=xT_r[:, cc:cc + 2, ts])
    wp_sb = big.tile([128, FT, C], bf16, tag="wp")
    wp_r = wp.rearrange("(k p) c -> p k c", p=128)
    for kk in range(FT):
        nc.sync.dma_start(out=wp_sb[:, kk], in_=wp_r[:, kk])

    # mask2: causal triangle duplicated side by side [128, 256]; ones8 for v
    mask2 = cmask_sb[:, 0:256].rearrange("p (two q) -> p two q", two=2)
    ones8 = cmask_sb[:, 256:264]

    # v storage: per 128-token tile, [128 tok, 8 heads, 64+1]; col 64 = ones
    # so each PV matmul's output row 64 accumulates the softmax denominators.
    v_tiles = []
    for tt in range(NKT):
        vt = v_pool.tile([128, H // 2, D + 1], bf16, tag=f"v{tt}")
        nc.vector.tensor_copy(vt[:, :, D], ones8[:])
        v_tiles.append(vt)

    qT = [qkt_pool.tile([128, T], bf16, tag=f"qT{hp}", name=f"qT{hp}")
          for hp in range(FT)]
    kT = [qkt_pool.tile([128, T], bf16, tag=f"kT{hp}", name=f"kT{hp}")
          for hp in range(FT)]
    oT = [ot_pool.tile([128, T], bf16, tag=f"oT{hp}", name=f"oT{hp}")
          for hp in range(FT)]

    # ---- emission helpers; each is one PE "filler" work unit ----
    def emit_v_group(s, t4):
        ts = slice(s * 512, (s + 1) * 512)
        tt = s * 4 + t4
        psv = ps_misc.tile([128, F], f32, tag="misc", name=f"psv{tt}")
        for cc in range(CCH):
            nc.tensor.matmul(psv[:],
                             x_sb[:, cc, ts][:, t4 * 128:(t4 + 1) * 128],
                             wv_sb[:, cc, :],
                             start=(cc == 0), stop=(cc == CCH - 1))
        nc.vector.tensor_copy(v_tiles[tt][:, :, 0:D],
                              psv.rearrange("p (h d) -> p h d", h=H // 2))

    def emit_qk_group(s, hp, which):
        ts = slice(s * 512, (s + 1) * 512)
        w_sb, dst, bi = ((wq_sb, qT, 0) if which == "q" else (wk_sb, kT, 1))
        ps = ps_misc.tile([128, 512], f32, tag="misc", name=f"ps{which}{s}{hp}")
        for cc in range(CCH):
            nc.tensor.matmul(ps[:], w_sb[:, hp, cc, :], x_sb[:, cc, ts],
                             start=(cc == 0), stop=(cc == CCH - 1))
        nc.vector.tensor_scalar_add(dst[hp][:, ts], ps[:],
                                    bqk_sb[:, bi, hp:hp + 1])

    def emit_y_group(j, t4, n):
        tt = 4 * j + t4
        psy = ps_misc.tile([128, 512], f32, tag="misc", name=f"psy{tt}{n}")
        for hp in range(FT):
            nc.tensor.matmul(
                psy[:], oT[hp][:, t4 * 128 + j * 512:t4 * 128 + j * 512 + 128],
                wp_sb[:, hp, n * 512:(n + 1) * 512],
                start=(hp == 0), stop=(hp == FT - 1))
        y_sb = tmp_pool.tile([128, 512], f32, tag="ysb")
        nc.vector.tensor_copy(y_sb[:], psy[:])
        for h in range(2):
            nc.sync.dma_start(
                out=y[tt * 128:(tt + 1) * 128,
                      n * 512 + h * 256:n * 512 + h * 256 + 256],
                in_=y_sb[:, h * 256:h * 256 + 256])

    def proj_fillers(s, first=False):
        fs = []
        if first:
            # q0/k0 first so stage-0 attention can start while wv streams in
            fs.append(lambda: emit_qk_group(s, 0, "q"))
            fs.append(lambda: emit_qk_group(s, 0, "k"))
        for t4 in range(4):
            fs.append(lambda t4=t4: emit_v_group(s, t4))
        for hp in range(0 if not first else 1, FT):
            fs.append(lambda hp=hp: emit_qk_group(s, hp, "q"))
            fs.append(lambda hp=hp: emit_qk_group(s, hp, "k"))
        return fs

    def y_fillers(j):
        return [
            (lambda t4=t4, n=n: emit_y_group(j, t4, n))
            for t4 in range(4) for n in range(2)
        ]

    # ---- prologue: projections for token block 0 ----
    for f in proj_fillers(0, first=True):
        f()

    # ---- stages: attention(j=s) with proj(s+1) and y(j<s-1) interleaved ----
    # Filler placement follows the per-stage ACT load: exp work grows with j
    # (nk = 4j+4 tiles) so late stages get the y-projection groups, keeping
    # the PE fed while ScalarE churns through the j=3 exp stream.
    stage_fillers = {
        0: lambda: proj_fillers(1),
        1: lambda: proj_fillers(2),
        2: lambda: proj_fillers(3),
        3: lambda: y_fillers(0) + y_fillers(1) + y_fillers(2),
    }
    for s in range(NQ):
        j = s
        nk = 4 * j + 4
        fillers = stage_fillers[s]()
        # in the last stage, hold back a few fillers so the PE has work
        # during the final normalize chain before the bare y(3) epilogue
        reserve = 3 if s == NQ - 1 else 0
        paced = len(fillers) - reserve
        total_iters = FT * nk
        it = fi = 0

        for hp in range(FT):
            o_ps = [ps_o.tile([128, 512], f32, tag="o", name=f"o{h2}")
                    for h2 in range(2)]

            def emit_s(i):
                # straddle tiles (r>0) only touch q >= 128*r within the
                # q-tile; the PSUM zero-fill from the i==0 start covers the
                # untouched (causally masked) columns.
                r = i - 4 * j
                qo = 128 * r if r > 0 else 0
                s2 = ps_s.tile([128, 1024], f32, tag="s", name=f"s2_{i%2}")
                for h2 in range(2):
                    lo = h2 * 64
                    nc.tensor.matmul(s2[:, 512 * h2 + qo:512 * h2 + 512],
                                     kT[hp][lo:lo + 64, i * 128:(i + 1) * 128],
                                     qT[hp][lo:lo + 64,
                                            j * 512 + qo:(j + 1) * 512],
                                     start=True, stop=True)
                # one exp for both heads across the 2-bank PSUM tile
                s2_r = s2.rearrange("p (two q) -> p two q", two=2)
                pt = pt_pool.tile([128, 1024], bf16, tag="pt")
                pt_r = pt.rearrange("p (two q) -> p two q", two=2)
                nc.scalar.activation(pt_r[:, :, qo:512], s2_r[:, :, qo:512],
                                     Exp, scale=SCALE)
                if r >= 0:
                    # causal edge: first 128 valid columns get the triangle
                    nc.vector.tensor_mul(pt_r[:, :, qo:qo + 128],
                                         pt_r[:, :, qo:qo + 128], mask2[:])
                return pt, qo

            def emit_pv(i, pt, qo):
                for h2 in range(2):
                    nc.tensor.matmul(o_ps[h2][0:D + 1, qo:512],
                                     v_tiles[i][:, 2 * hp + h2, :],
                                     pt[:, 512 * h2 + qo:512 * h2 + 512],
                                     start=(i == 0), stop=(i == nk - 1))

            # Cluster S pairs two at a time (exactly the ps_s ring capacity):
            # the second pair's row-0 ldweights hides under the first pair's
            # rows-64-127 matmul, and the PV pairs then chain back-to-back.
            for m in range(0, nk, 2):
                a = emit_s(m)
                b = emit_s(m + 1)
                emit_pv(m, *a)
                emit_pv(m + 1, *b)
                it += 2
                while fi * total_iters < paced * it:
                    fillers[fi]()
                    fi += 1
            # ---- normalize: divide rows 0..63 by the sums row (64) ----
            # baseline-proven sequence: copy sums to SBUF lane 64, DMA to
            # partition 0, reciprocal there, gpsimd-broadcast, multiply.
            sums = norm_pool.tile([65, 1024], f32, tag="sums")
            nc.vector.tensor_copy(sums[64:65, 0:512], o_ps[0][64:65, 0:512])
            nc.vector.tensor_copy(sums[64:65, 512:1024],
                                  o_ps[1][64:65, 0:512])
            sums_lo = norm_pool.tile([1, 1024], f32, tag="sums_lo")
            nc.sync.dma_start(out=sums_lo[0:1, :], in_=sums[64:65, :])
            recs = norm_pool.tile([1, 1024], f32, tag="recs")
            nc.vector.reciprocal_approx_fast(recs[0:1, :], sums_lo[0:1, :])
            bc_e = norm_pool.tile([64, 512], f32, tag="bc_e")
            nc.gpsimd.partition_broadcast(bc_e[:], recs[0:1, 0:512],
                                          channels=64)
            bc_o = norm_pool.tile([64, 512], f32, tag="bc_o")
            nc.gpsimd.partition_broadcast(bc_o[:], recs[0:1, 512:1024],
                                          channels=64)
            nc.vector.tensor_mul(oT[hp][0:64, j * 512:(j + 1) * 512],
                                 o_ps[0][0:64, :], bc_e[:])
            tmp = tmp_pool.tile([64, 512], bf16, tag="tmp")
            nc.vector.tensor_mul(tmp[:], o_ps[1][0:64, :], bc_o[:])
            for h in range(2):
                cs = slice(j * 512 + h * 256, j * 512 + h * 256 + 256)
                nc.sync.dma_start(out=oT[hp][64:128, cs],
                                  in_=tmp[:, h * 256:h * 256 + 256])
            if fi < len(fillers):
                fillers[fi]()
                fi += 1
            if dbg is not None and hp == 0 and j == 0:
                nc.sync.dma_start(out=dbg["d_recs0"][:], in_=recs[0:1, :])
        while fi < len(fillers):
            fillers[fi]()
            fi += 1

    # ---- epilogue: final q-tile's output projection ----
    for f in y_fillers(NQ - 1):
        f()

    if dbg is not None:
        nc.sync.dma_start(out=dbg["d_qT0"][:], in_=qT[0][:])
        nc.sync.dma_start(out=dbg["d_kT0"][:], in_=kT[0][:])
        nc.sync.dma_start(out=dbg["d_v0"][:], in_=v_tiles[0][:])
        for hp in range(FT):
            nc.sync.dma_start(out=dbg[f"d_oT{hp}"][:], in_=oT[hp][:])

    for p in reversed(pools):
        p.release()


def make_in_maps(x, Wq, bq, Wk, bk, Wv, bv, Wp, bp):
    x = np.asarray(x, dtype=np.float32)
    Wq, Wk, Wv, Wp = (np.asarray(a, dtype=np.float32) for a in (Wq, Wk, Wv, Wp))
    bq, bk, bv, bp = (np.asarray(a, dtype=np.float32) for a in (bq, bk, bv, bp))
    b16 = ml_dtypes.bfloat16
    in_maps = []
    for g in range(N_CORES):
        b, half = g // 2, g % 2
        fs = slice(half * F, (half + 1) * F)
        # [C, 128f] -> [hp, p, k, ff] with c = k*128 + p, f = hp*128 + ff
        def shuf(w):
            return np.ascontiguousarray(
                w[:, fs].reshape(CCH, 128, FT, 128).transpose(2, 1, 0, 3)
                .astype(b16))
        in_maps.append({
            "xT": np.ascontiguousarray(x[b].T.astype(b16)),
            "wq": shuf(Wq),
            "wk": shuf(Wk),
            "wv": np.ascontiguousarray(Wv[:, fs].astype(b16)),
            "wp": np.ascontiguousarray(Wp[fs, :].astype(b16)),
            "bqk": np.ascontiguousarray(np.stack([bq[fs], bk[fs]])),
            "cmask": _cmask(),
        })
    return in_maps


def _cmask():
    if "cmask" not in _cache:
        q = np.arange(128, dtype=np.float64)[None, :]
        kk = np.arange(128, dtype=np.float64)[:, None]
        tri = (q >= kk).astype(np.float32)
        c = np.concatenate([tri, tri, np.ones((128, 8), np.float32)], axis=1)
        _cache["cmask"] = np.ascontiguousarray(c.astype(ml_dtypes.bfloat16))
    return _cache["cmask"]


def gather(results, bv, Wv, Wp, bp):
    bias_total = (np.asarray(bv, np.float32) @ np.asarray(Wp, np.float32)
                  + np.asarray(bp, np.float32))
    y = np.empty((B, T, C), dtype=np.float32)
    for b in range(B):
        y[b] = results[2 * b]["y"] + results[2 * b + 1]["y"] + bias_total
    return y


def get_nc():
    if "nc" not in _cache:
        _cache["nc"] = _build()
    return _cache["nc"]


def kernel(x, Wq, bq, Wk, bk, Wv, bv, Wp, bp):
    nc = get_nc()
    in_maps = make_in_maps(x, Wq, bq, Wk, bk, Wv, bv, Wp, bp)
    res = run_bass_kernel_spmd(nc, in_maps, list(range(N_CORES)))
    return gather(res.results, bv, Wv, Wp, bp)



# revision 25
# speedup vs baseline: 1.0037x; 1.0037x over previous
"""Causal self-attention (B=4, T=2048, C=1024, H=16) on 8 TRN2 NeuronCores.

Sharding: data-parallel over batch (4) x tensor-parallel over head-halves (2).
Core g handles batch g//2 and heads [8*(g%2), 8*(g%2)+8). Megatron-style:
Wq/Wk/Wv column-sharded, Wp row-sharded; the host sums the two partial y
contributions per batch and adds the (bv @ Wp + bp) term (valid because
softmax rows sum to 1, so the v-bias passes through attention).

v2 design (vs the f32r baseline):
  - all matmul operands bf16 (inputs converted host-side): halves DMA/SBUF,
    enables FWL weight loads, no narrow-N f32r penalty on straddle tiles.
  - x^T fully resident in SBUF; projections re-read it from SBUF instead of
    re-streaming 8MB from HBM per head-pair.
  - stage pipeline: for s in 0..3: project(tq=s) -> attention(j=s) -> y(j=s).
    Causality makes attention j=s depend only on projections tq<=s, so the
    PE stays dense and the exp stream starts ~20us into the kernel.
  - S-pair matmuls (two heads row-packed at partitions 0/64) write one
    2-bank PSUM tile [128,1024]; ONE ScalarE exp covers both heads, halving
    ACT instruction count.
  - normalize without gpsimd: reciprocal of the sums rows straight from
    PSUM (lane 64), broadcast to 64 partitions via K=1 PE matmuls, then DVE
    multiplies. Odd head still needs one small SBUF->SBUF DMA shift.
  - y projection accumulates in PSUM and DMAs straight to HBM (no copy).
"""

import math

import numpy as np
import ml_dtypes

import concourse.bass as bass
import concourse.tile as tile
from concourse import bacc, mybir
from concourse.bass_utils import run_bass_kernel_spmd

B, T, C, H = 4, 2048, 1024, 16
D = C // H  # 64
N_CORES = 8
F = C // 2  # 512 features per core (8 heads)
FT = F // 128  # 4 feature tiles (head pairs) per core
CCH = C // 128  # 8 contraction chunks
NQ = T // 512  # 4 q-tiles / stages
NKT = T // 128  # 16 k-tiles
SCALE = 1.0 / math.sqrt(D)

f32 = mybir.dt.float32
f32r = mybir.dt.float32r
bf16 = mybir.dt.bfloat16

_cache = {}
DEBUG_DUMPS = False


def _build():
    nc = bacc.Bacc("TRN2", target_bir_lowering=False, debug=False,
                   num_devices=N_CORES)
    # All inputs are stored partition-major host-side so every DMA line is a
    # contiguous >=1KB segment per partition (descriptor-efficient).
    xT = nc.dram_tensor("xT", [128, CCH, T], bf16, kind="ExternalInput").ap()
    wq = nc.dram_tensor("wq", [128, FT, CCH, 128], bf16, kind="ExternalInput").ap()
    wk = nc.dram_tensor("wk", [128, FT, CCH, 128], bf16, kind="ExternalInput").ap()
    wv = nc.dram_tensor("wv", [128, CCH, F], bf16, kind="ExternalInput").ap()
    wp = nc.dram_tensor("wp", [128, FT, C], bf16, kind="ExternalInput").ap()
    bqk = nc.dram_tensor("bqk", [128, 2, FT], f32, kind="ExternalInput").ap()
    cmask = nc.dram_tensor("cmask", [128, 264], bf16, kind="ExternalInput").ap()
    y = nc.dram_tensor("y", [T, C], f32, kind="ExternalOutput").ap()
    dbg = None
    if DEBUG_DUMPS:
        dbg = {
            "d_qT0": nc.dram_tensor("d_qT0", [128, T], bf16,
                                    kind="ExternalOutput").ap(),
            "d_kT0": nc.dram_tensor("d_kT0", [128, T], bf16,
                                    kind="ExternalOutput").ap(),
            "d_v0": nc.dram_tensor("d_v0", [128, H // 2, D + 1], bf16,
                                   kind="ExternalOutput").ap(),
            "d_pt00": nc.dram_tensor("d_pt00", [128, 1024], bf16,
                                     kind="ExternalOutput").ap(),
            "d_oT0": nc.dram_tensor("d_oT0", [128, T], bf16,
                                    kind="ExternalOutput").ap(),
            "d_oT1": nc.dram_tensor("d_oT1", [128, T], bf16,
                                    kind="ExternalOutput").ap(),
            "d_oT2": nc.dram_tensor("d_oT2", [128, T], bf16,
                                    kind="ExternalOutput").ap(),
            "d_oT3": nc.dram_tensor("d_oT3", [128, T], bf16,
                                    kind="ExternalOutput").ap(),
            "d_recs0": nc.dram_tensor("d_recs0", [1, 1024], f32,
                                      kind="ExternalOutput").ap(),
        }

    with tile.TileContext(nc) as tc:
        _body(tc, xT, wq, wk, wv, wp, bqk, cmask, y, dbg)
    nc.compile()
    return nc


def _body(tc, xT, wq, wk, wv, wp, bqk, cmask, y, dbg=None):
    nc = tc.nc
    Exp = mybir.ActivationFunctionType.Exp

    pools = []

    def pool(**kw):
        p = tc.alloc_tile_pool(**kw)
        pools.append(p)
        return p

    consts = pool(name="consts", bufs=1)
    big = pool(name="big", bufs=1)
    qkt_pool = pool(name="qkt", bufs=1)
    v_pool = pool(name="v", bufs=1)
    ot_pool = pool(name="ot", bufs=1)
    pt_pool = pool(name="pt", bufs=8)
    norm_pool = pool(name="norm", bufs=2)
    tmp_pool = pool(name="tmp", bufs=2)
    ps_misc = pool(name="ps_misc", bufs=2, space="PSUM")
    ps_s = pool(name="ps_s", bufs=2, space="PSUM")
    ps_o = pool(name="ps_o", bufs=2, space="PSUM")

    # ---- resident inputs, DMA'd in the order the first stage needs ----
    cmask_sb = consts.tile([128, 264], bf16, tag="cmask")
    for h in range(2):
        nc.sync.dma_start(out=cmask_sb[:, 132 * h:132 * h + 132],
                          in_=cmask[:, 132 * h:132 * h + 132])
    bqk_sb = consts.tile([128, 2, FT], f32, tag="bqk")
    nc.sync.dma_start(out=bqk_sb[:], in_=bqk[:])
    # startup transfers split per chunk so they spread round-robin across the
    # 16 DMA queues; a single big dma_start serializes on one queue.
    x_sb = big.tile([128, CCH, T], bf16, tag="x")
    for cc in range(CCH):
        nc.sync.dma_start(out=x_sb[:, cc, 0:512], in_=xT[:, cc, 0:512])
    wq_sb = big.tile([128, FT, CCH, 128], bf16, tag="wq")
    wk_sb = big.tile([128, FT, CCH, 128], bf16, tag="wk")
    for half in range(2):
        cs = slice(half * 4, half * 4 + 4)
        nc.sync.dma_start(out=wq_sb[:, 0, cs], in_=wq[:, 0, cs])
        nc.sync.dma_start(out=wk_sb[:, 0, cs], in_=wk[:, 0, cs])
    wv_sb = big.tile([128, CCH, F], bf16, tag="wv")
    for cc in range(0, CCH, 2):
        nc.sync.dma_start(out=wv_sb[:, cc:cc + 2], in_=wv[:, cc:cc + 2])
    for hp in range(1, FT):
        nc.sync.dma_start(out=wq_sb[:, hp], in_=wq[:, hp])
        nc.sync.dma_start(out=wk_sb[:, hp], in_=wk[:, hp])
    for tq in range(1, NQ):
        ts = slice(tq * 512, (tq + 1) * 512)
        for cc in range(CCH):
            nc.sync.dma_start(out=x_sb[:, cc, ts], in_=xT[:, cc, ts])
    wp_sb = big.tile([128, FT, C], bf16, tag="wp")
    for kk in range(FT):
        nc.sync.dma_start(out=wp_sb[:, kk], in_=wp[:, kk])

    # mask2: causal triangle duplicated side by side [128, 256]; ones8 for v
    mask2 = cmask_sb[:, 0:256].rearrange("p (two q) -> p two q", two=2)
    ones8 = cmask_sb[:, 256:264]

    # v storage: per 128-token tile, [128 tok, 8 heads, 64+1]; col 64 = ones
    # so each PV matmul's output row 64 accumulates the softmax denominators.
    v_tiles = []
    for tt in range(NKT):
        vt = v_pool.tile([128, H // 2, D + 1], bf16, tag=f"v{tt}")
        nc.vector.tensor_copy(vt[:, :, D], ones8[:])
        v_tiles.append(vt)

    qT = [qkt_pool.tile([128, T], bf16, tag=f"qT{hp}", name=f"qT{hp}")
          for hp in range(FT)]
    kT = [qkt_pool.tile([128, T], bf16, tag=f"kT{hp}", name=f"kT{hp}")
          for hp in range(FT)]
    oT = [ot_pool.tile([128, T], bf16, tag=f"oT{hp}", name=f"oT{hp}")
          for hp in range(FT)]

    # ---- emission helpers; each is one PE "filler" work unit ----
    def emit_v_group(s, t4):
        ts = slice(s * 512, (s + 1) * 512)
        tt = s * 4 + t4
        psv = ps_misc.tile([128, F], f32, tag="misc", name=f"psv{tt}")
        for cc in range(CCH):
            nc.tensor.matmul(psv[:],
                             x_sb[:, cc, ts][:, t4 * 128:(t4 + 1) * 128],
                             wv_sb[:, cc, :],
                             start=(cc == 0), stop=(cc == CCH - 1))
        nc.vector.tensor_copy(v_tiles[tt][:, :, 0:D],
                              psv.rearrange("p (h d) -> p h d", h=H // 2))

    def emit_qk_group(s, hp, which):
        ts = slice(s * 512, (s + 1) * 512)
        w_sb, dst, bi = ((wq_sb, qT, 0) if which == "q" else (wk_sb, kT, 1))
        ps = ps_misc.tile([128, 512], f32, tag="misc", name=f"ps{which}{s}{hp}")
        for cc in range(CCH):
            nc.tensor.matmul(ps[:], w_sb[:, hp, cc, :], x_sb[:, cc, ts],
                             start=(cc == 0), stop=(cc == CCH - 1))
        nc.vector.tensor_scalar_add(dst[hp][:, ts], ps[:],
                                    bqk_sb[:, bi, hp:hp + 1])

    def emit_y_group(j, t4, n):
        tt = 4 * j + t4
        psy = ps_misc.tile([128, 512], f32, tag="misc", name=f"psy{tt}{n}")
        for hp in range(FT):
            nc.tensor.matmul(
                psy[:], oT[hp][:, t4 * 128 + j * 512:t4 * 128 + j * 512 + 128],
                wp_sb[:, hp, n * 512:(n + 1) * 512],
                start=(hp == 0), stop=(hp == FT - 1))
        y_sb = tmp_pool.tile([128, 512], f32, tag="ysb")
        nc.vector.tensor_copy(y_sb[:], psy[:])
        for h in range(2):
            nc.sync.dma_start(
                out=y[tt * 128:(tt + 1) * 128,
                      n * 512 + h * 256:n * 512 + h * 256 + 256],
                in_=y_sb[:, h * 256:h * 256 + 256])

    def proj_fillers(s, first=False):
        fs = []
        if first:
            # q0/k0 first so stage-0 attention can start while wv streams in
            fs.append(lambda: emit_qk_group(s, 0, "q"))
            fs.append(lambda: emit_qk_group(s, 0, "k"))
        for t4 in range(4):
            fs.append(lambda t4=t4: emit_v_group(s, t4))
        for hp in range(0 if not first else 1, FT):
            fs.append(lambda hp=hp: emit_qk_group(s, hp, "q"))
            fs.append(lambda hp=hp: emit_qk_group(s, hp, "k"))
        return fs

    def y_fillers(j):
        return [
            (lambda t4=t4, n=n: emit_y_group(j, t4, n))
            for t4 in range(4) for n in range(2)
        ]

    # ---- HAM warm-up: ~3.5us of throwaway matmuls on the (early-arriving)
    # cmask tile so the PE clock gate is already at 8/8 when the first real
    # projection data lands; otherwise the first ~14 matmuls run at 1.2GHz.
    warm_ps = ps_misc.tile([128, 256], f32, tag="misc", name="warm")
    for _ in range(18):
        nc.tensor.matmul(warm_ps[:], cmask_sb[:, 0:128], cmask_sb[:, 0:256],
                         start=True, stop=True)

    # ---- prologue: projections for token block 0 ----
    for f in proj_fillers(0, first=True):
        f()

    # ---- stages: attention(j=s) with proj(s+1) and y(j<s-1) interleaved ----
    # Filler placement follows the per-stage ACT load: exp work grows with j
    # (nk = 4j+4 tiles) so late stages get the y-projection groups, keeping
    # the PE fed while ScalarE churns through the j=3 exp stream.
    stage_fillers = {
        0: lambda: proj_fillers(1),
        1: lambda: proj_fillers(2),
        2: lambda: proj_fillers(3),
        3: lambda: y_fillers(0) + y_fillers(1) + y_fillers(2),
    }

    def pair_up(fs):
        # emit fillers two groups at a time: consecutive groups chain on the
        # PE (second group's ldweights hides under the first group's last
        # matmul), halving the attention->filler transition stalls
        def mk(i):
            def run():
                fs[i]()
                if i + 1 < len(fs):
                    fs[i + 1]()
            return run
        return [mk(i) for i in range(0, len(fs), 2)]
    for s in range(NQ):
        j = s
        nk = 4 * j + 4
        fillers = pair_up(stage_fillers[s]())
        total_iters = FT * nk
        it = fi = 0

        for hp in range(FT):
            o_ps = [ps_o.tile([128, 512], f32, tag="o", name=f"o{h2}")
                    for h2 in range(2)]

            def emit_s(i):
                # straddle tiles (r>0) only touch q >= 128*r within the
                # q-tile; the PSUM zero-fill from the i==0 start covers the
                # untouched (causally masked) columns.
                r = i - 4 * j
                qo = 128 * r if r > 0 else 0
                s2 = ps_s.tile([128, 1024], f32, tag="s", name=f"s2_{i%2}")
                for h2 in range(2):
                    lo = h2 * 64
                    nc.tensor.matmul(s2[:, 512 * h2 + qo:512 * h2 + 512],
                                     kT[hp][lo:lo + 64, i * 128:(i + 1) * 128],
                                     qT[hp][lo:lo + 64,
                                            j * 512 + qo:(j + 1) * 512],
                                     start=True, stop=True)
                # one exp for both heads across the 2-bank PSUM tile
                s2_r = s2.rearrange("p (two q) -> p two q", two=2)
                pt = pt_pool.tile([128, 1024], bf16, tag="pt")
                pt_r = pt.rearrange("p (two q) -> p two q", two=2)
                nc.scalar.activation(pt_r[:, :, qo:512], s2_r[:, :, qo:512],
                                     Exp, scale=SCALE)
                if r >= 0:
                    # causal edge: first 128 valid columns get the triangle
                    nc.vector.tensor_mul(pt_r[:, :, qo:qo + 128],
                                         pt_r[:, :, qo:qo + 128], mask2[:])
                return pt, qo

            def emit_pv(i, pt, qo):
                for h2 in range(2):
                    nc.tensor.matmul(o_ps[h2][0:D + 1, qo:512],
                                     v_tiles[i][:, 2 * hp + h2, :],
                                     pt[:, 512 * h2 + qo:512 * h2 + 512],
                                     start=(i == 0), stop=(i == nk - 1))

            # Software-pipeline S ahead of PV by one tile: exp(i) then runs
            # while the PE does PV(i-1) + fillers, so PV(i) never waits on
            # the ScalarE exp stream.
            pending = None
            for i in range(nk):
                cur = (i, *emit_s(i))
                if pending is not None:
                    emit_pv(*pending)
                pending = cur
                it += 1
                while fi * total_iters < len(fillers) * it:
                    fillers[fi]()
                    fi += 1
            emit_pv(*pending)
            # ---- normalize: divide rows 0..63 by the sums row (64) ----
            sums = norm_pool.tile([65, 1024], f32, tag="sums")
            nc.vector.tensor_copy(sums[64:65, 0:512], o_ps[0][64:65, 0:512])
            nc.vector.tensor_copy(sums[64:65, 512:1024],
                                  o_ps[1][64:65, 0:512])
            # (a K=1 PE-matmul broadcast from partition 64 passes CoreSim but
            # produces garbage on hardware — keep the proven DMA+gpsimd path)
            sums_lo = norm_pool.tile([1, 1024], f32, tag="sums_lo")
            for h in range(2):
                nc.sync.dma_start(out=sums_lo[0:1, 512 * h:512 * h + 512],
                                  in_=sums[64:65, 512 * h:512 * h + 512])
            recs = norm_pool.tile([1, 1024], f32, tag="recs")
            nc.vector.reciprocal_approx_fast(recs[0:1, :], sums_lo[0:1, :])
            bc_e = norm_pool.tile([64, 512], f32, tag="bc_e")
            nc.gpsimd.partition_broadcast(bc_e[:], recs[0:1, 0:512],
                                          channels=64)
            bc_o = norm_pool.tile([64, 512], f32, tag="bc_o")
            nc.gpsimd.partition_broadcast(bc_o[:], recs[0:1, 512:1024],
                                          channels=64)
            nc.vector.tensor_mul(oT[hp][0:64, j * 512:(j + 1) * 512],
                                 o_ps[0][0:64, :], bc_e[:])
            tmp = tmp_pool.tile([64, 512], bf16, tag="tmp")
            nc.vector.tensor_mul(tmp[:], o_ps[1][0:64, :], bc_o[:])
            for h in range(2):
                cs = slice(j * 512 + h * 256, j * 512 + h * 256 + 256)
                nc.sync.dma_start(out=oT[hp][64:128, cs],
                                  in_=tmp[:, h * 256:h * 256 + 256])
            if fi < len(fillers):
                fillers[fi]()
                fi += 1
            if dbg is not None and hp == 0 and j == 0:
                nc.sync.dma_start(out=dbg["d_recs0"][:], in_=recs[0:1, :])
        while fi < len(fillers):
            fillers[fi]()
            fi += 1

    # ---- epilogue: final q-tile's output projection ----
    for f in y_fillers(NQ - 1):
        f()

    if dbg is not None:
        nc.sync.dma_start(out=dbg["d_qT0"][:], in_=qT[0][:])
        nc.sync.dma_start(out=dbg["d_kT0"][:], in_=kT[0][:])
        nc.sync.dma_start(out=dbg["d_v0"][:], in_=v_tiles[0][:])
        for hp in range(FT):
            nc.sync.dma_start(out=dbg[f"d_oT{hp}"][:], in_=oT[hp][:])

    for p in reversed(pools):
        p.release()


def make_in_maps(x, Wq, bq, Wk, bk, Wv, bv, Wp, bp):
    x = np.asarray(x, dtype=np.float32)
    Wq, Wk, Wv, Wp = (np.asarray(a, dtype=np.float32) for a in (Wq, Wk, Wv, Wp))
    bq, bk, bv, bp = (np.asarray(a, dtype=np.float32) for a in (bq, bk, bv, bp))
    b16 = ml_dtypes.bfloat16
    in_maps = []
    for g in range(N_CORES):
        b, half = g // 2, g % 2
        fs = slice(half * F, (half + 1) * F)
        # [C, 128f] -> [p, hp, k, ff] with c = k*128 + p, f = hp*128 + ff
        def shuf(w):
            return np.ascontiguousarray(
                w[:, fs].reshape(CCH, 128, FT, 128).transpose(1, 2, 0, 3)
                .astype(b16))
        # bqk[p, b, hp] = bias_b[hp*128 + p]
        bqk_h = np.ascontiguousarray(
            np.stack([bq[fs], bk[fs]]).reshape(2, FT, 128)
            .transpose(2, 0, 1))
        in_maps.append({
            # xT[p, k, t] = x[b][t, k*128+p]
            "xT": np.ascontiguousarray(
                x[b].T.reshape(CCH, 128, T).transpose(1, 0, 2).astype(b16)),
            "wq": shuf(Wq),
            "wk": shuf(Wk),
            # wv[p, k, f] = Wv[k*128+p, fs][f]
            "wv": np.ascontiguousarray(
                Wv[:, fs].reshape(CCH, 128, F).transpose(1, 0, 2).astype(b16)),
            # wp[p, k, c] = Wp[fs][k*128+p, c]
            "wp": np.ascontiguousarray(
                Wp[fs, :].reshape(FT, 128, C).transpose(1, 0, 2).astype(b16)),
            "bqk": bqk_h,
            "cmask": _cmask(),
        })
    return in_maps


def _cmask():
    if "cmask" not in _cache:
        q = np.arange(128, dtype=np.float64)[None, :]
        kk = np.arange(128, dtype=np.float64)[:, None]
        tri = (q >= kk).astype(np.float32)
        c = np.concatenate([tri, tri, np.ones((128, 8), np.float32)], axis=1)
        _cache["cmask"] = np.ascontiguousarray(c.astype(ml_dtypes.bfloat16))
    return _cache["cmask"]


def gather(results, bv, Wv, Wp, bp):
    bias_total = (np.asarray(bv, np.float32) @ np.asarray(Wp, np.float32)
                  + np.asarray(bp, np.float32))
    y = np.empty((B, T, C), dtype=np.float32)
    for b in range(B):
        y[b] = results[2 * b]["y"] + results[2 * b + 1]["y"] + bias_total
    return y


def get_nc():
    if "nc" not in _cache:
        _cache["nc"] = _build()
    return _cache["nc"]


def kernel(x, Wq, bq, Wk, bk, Wv, bv, Wp, bp):
    nc = get_nc()
    in_maps = make_in_maps(x, Wq, bq, Wk, bk, Wv, bv, Wp, bp)
    res = run_bass_kernel_spmd(nc, in_maps, list(range(N_CORES)))
    return gather(res.results, bv, Wv, Wp, bp)



# revision 30
# speedup vs baseline: 1.0287x; 1.0249x over previous
"""Causal self-attention (B=4, T=2048, C=1024, H=16) on 8 TRN2 NeuronCores.

Sharding: data-parallel over batch (4) x tensor-parallel over head-halves (2).
Core g handles batch g//2 and heads [8*(g%2), 8*(g%2)+8). Megatron-style:
Wq/Wk/Wv column-sharded, Wp row-sharded; the host sums the two partial y
contributions per batch and adds the (bv @ Wp + bp) term (valid because
softmax rows sum to 1, so the v-bias passes through attention).

v2 design (vs the f32r baseline):
  - all matmul operands bf16 (inputs converted host-side): halves DMA/SBUF,
    enables FWL weight loads, no narrow-N f32r penalty on straddle tiles.
  - x^T fully resident in SBUF; projections re-read it from SBUF instead of
    re-streaming 8MB from HBM per head-pair.
  - stage pipeline: for s in 0..3: project(tq=s) -> attention(j=s) -> y(j=s).
    Causality makes attention j=s depend only on projections tq<=s, so the
    PE stays dense and the exp stream starts ~20us into the kernel.
  - S-pair matmuls (two heads row-packed at partitions 0/64) write one
    2-bank PSUM tile [128,1024]; ONE ScalarE exp covers both heads, halving
    ACT instruction count.
  - normalize without gpsimd: reciprocal of the sums rows straight from
    PSUM (lane 64), broadcast to 64 partitions via K=1 PE matmuls, then DVE
    multiplies. Odd head still needs one small SBUF->SBUF DMA shift.
  - y projection accumulates in PSUM and DMAs straight to HBM (no copy).
"""

import math

import numpy as np
import ml_dtypes

import concourse.bass as bass
import concourse.tile as tile
from concourse import bacc, mybir
from concourse.bass_utils import run_bass_kernel_spmd

B, T, C, H = 4, 2048, 1024, 16
D = C // H  # 64
N_CORES = 8
F = C // 2  # 512 features per core (8 heads)
FT = F // 128  # 4 feature tiles (head pairs) per core
CCH = C // 128  # 8 contraction chunks
NQ = T // 512  # 4 q-tiles / stages
NKT = T // 128  # 16 k-tiles
SCALE = 1.0 / math.sqrt(D)

f32 = mybir.dt.float32
f32r = mybir.dt.float32r
bf16 = mybir.dt.bfloat16

_cache = {}
DEBUG_DUMPS = False


def _build():
    nc = bacc.Bacc("TRN2", target_bir_lowering=False, debug=False,
                   num_devices=N_CORES)
    # All inputs are stored partition-major host-side so every DMA line is a
    # contiguous >=1KB segment per partition (descriptor-efficient).
    xT = nc.dram_tensor("xT", [128, CCH, T], bf16, kind="ExternalInput").ap()
    wq = nc.dram_tensor("wq", [128, FT, CCH, 128], bf16, kind="ExternalInput").ap()
    wk = nc.dram_tensor("wk", [128, FT, CCH, 128], bf16, kind="ExternalInput").ap()
    wv = nc.dram_tensor("wv", [128, CCH, F], bf16, kind="ExternalInput").ap()
    wp = nc.dram_tensor("wp", [128, FT, C], bf16, kind="ExternalInput").ap()
    bqk = nc.dram_tensor("bqk", [128, 2, FT], f32, kind="ExternalInput").ap()
    cmask = nc.dram_tensor("cmask", [128, 264], bf16, kind="ExternalInput").ap()
    y = nc.dram_tensor("y", [T, C], f32, kind="ExternalOutput").ap()
    dbg = None
    if DEBUG_DUMPS:
        dbg = {
            "d_qT0": nc.dram_tensor("d_qT0", [128, T], bf16,
                                    kind="ExternalOutput").ap(),
            "d_kT0": nc.dram_tensor("d_kT0", [128, T], bf16,
                                    kind="ExternalOutput").ap(),
            "d_v0": nc.dram_tensor("d_v0", [128, H // 2, D + 1], bf16,
                                   kind="ExternalOutput").ap(),
            "d_pt00": nc.dram_tensor("d_pt00", [128, 1024], bf16,
                                     kind="ExternalOutput").ap(),
            "d_oT0": nc.dram_tensor("d_oT0", [128, T], bf16,
                                    kind="ExternalOutput").ap(),
            "d_oT1": nc.dram_tensor("d_oT1", [128, T], bf16,
                                    kind="ExternalOutput").ap(),
            "d_oT2": nc.dram_tensor("d_oT2", [128, T], bf16,
                                    kind="ExternalOutput").ap(),
            "d_oT3": nc.dram_tensor("d_oT3", [128, T], bf16,
                                    kind="ExternalOutput").ap(),
            "d_recs0": nc.dram_tensor("d_recs0", [1, 1024], f32,
                                      kind="ExternalOutput").ap(),
        }

    with tile.TileContext(nc) as tc:
        _body(tc, xT, wq, wk, wv, wp, bqk, cmask, y, dbg)
    nc.compile()
    return nc


def _body(tc, xT, wq, wk, wv, wp, bqk, cmask, y, dbg=None):
    nc = tc.nc
    Exp = mybir.ActivationFunctionType.Exp

    pools = []

    def pool(**kw):
        p = tc.alloc_tile_pool(**kw)
        pools.append(p)
        return p

    consts = pool(name="consts", bufs=1)
    big = pool(name="big", bufs=1)
    qkt_pool = pool(name="qkt", bufs=1)
    v_pool = pool(name="v", bufs=1)
    ot_pool = pool(name="ot", bufs=1)
    pt_pool = pool(name="pt", bufs=8)
    norm_pool = pool(name="norm", bufs=2)
    tmp_pool = pool(name="tmp", bufs=2)
    ps_misc = pool(name="ps_misc", bufs=2, space="PSUM")
    ps_s = pool(name="ps_s", bufs=2, space="PSUM")
    ps_o = pool(name="ps_o", bufs=2, space="PSUM")

    # ---- resident inputs, DMA'd in the order the first stage needs ----
    # startup transfers split per chunk so they spread round-robin across the
    # 16 DMA queues; a single big dma_start serializes on one queue. cmask
    # (which gates the PE warm-up) goes first, then the stage-0 x block and
    # the first q/k weight tiles which gate the first real projection.
    cmask_sb = consts.tile([128, 264], bf16, tag="cmask")
    nc.sync.dma_start(out=cmask_sb[:], in_=cmask[:])
    x_sb = big.tile([128, CCH, T], bf16, tag="x")
    for cc in range(CCH):
        nc.sync.dma_start(out=x_sb[:, cc, 0:512], in_=xT[:, cc, 0:512])
    wq_sb = big.tile([128, FT, CCH, 128], bf16, tag="wq")
    wk_sb = big.tile([128, FT, CCH, 128], bf16, tag="wk")
    for half in range(2):
        cs = slice(half * 4, half * 4 + 4)
        nc.sync.dma_start(out=wq_sb[:, 0, cs], in_=wq[:, 0, cs])
        nc.sync.dma_start(out=wk_sb[:, 0, cs], in_=wk[:, 0, cs])
    bqk_sb = consts.tile([128, 2, FT], f32, tag="bqk")
    nc.sync.dma_start(out=bqk_sb[:], in_=bqk[:])
    wv_sb = big.tile([128, CCH, F], bf16, tag="wv")
    for cc in range(0, CCH, 2):
        nc.sync.dma_start(out=wv_sb[:, cc:cc + 2], in_=wv[:, cc:cc + 2])
    for hp in range(1, FT):
        nc.sync.dma_start(out=wq_sb[:, hp], in_=wq[:, hp])
        nc.sync.dma_start(out=wk_sb[:, hp], in_=wk[:, hp])
    for tq in range(1, NQ):
        ts = slice(tq * 512, (tq + 1) * 512)
        for cc in range(CCH):
            nc.sync.dma_start(out=x_sb[:, cc, ts], in_=xT[:, cc, ts])
    wp_sb = big.tile([128, FT, C], bf16, tag="wp")
    for kk in range(FT):
        nc.sync.dma_start(out=wp_sb[:, kk], in_=wp[:, kk])

    # mask2: causal triangle duplicated side by side [128, 256]; ones8 for v
    mask2 = cmask_sb[:, 0:256].rearrange("p (two q) -> p two q", two=2)
    ones8 = cmask_sb[:, 256:264]

    # v storage: per 128-token tile, [128 tok, 8 heads, 64+1]; col 64 = ones
    # so each PV matmul's output row 64 accumulates the softmax denominators.
    v_tiles = []
    for tt in range(NKT):
        vt = v_pool.tile([128, H // 2, D + 1], bf16, tag=f"v{tt}")
        nc.vector.tensor_copy(vt[:, :, D], ones8[:])
        v_tiles.append(vt)

    qT = [qkt_pool.tile([128, T], bf16, tag=f"qT{hp}", name=f"qT{hp}")
          for hp in range(FT)]
    kT = [qkt_pool.tile([128, T], bf16, tag=f"kT{hp}", name=f"kT{hp}")
          for hp in range(FT)]
    oT = [ot_pool.tile([128, T], bf16, tag=f"oT{hp}", name=f"oT{hp}")
          for hp in range(FT)]

    # ---- emission helpers; each is one PE "filler" work unit ----
    def emit_v_group(s, t4):
        ts = slice(s * 512, (s + 1) * 512)
        tt = s * 4 + t4
        psv = ps_misc.tile([128, F], f32, tag="misc", name=f"psv{tt}")
        for cc in range(CCH):
            nc.tensor.matmul(psv[:],
                             x_sb[:, cc, ts][:, t4 * 128:(t4 + 1) * 128],
                             wv_sb[:, cc, :],
                             start=(cc == 0), stop=(cc == CCH - 1))
        nc.vector.tensor_copy(v_tiles[tt][:, :, 0:D],
                              psv.rearrange("p (h d) -> p h d", h=H // 2))

    def emit_qk_group(s, hp, which):
        ts = slice(s * 512, (s + 1) * 512)
        w_sb, dst, bi = ((wq_sb, qT, 0) if which == "q" else (wk_sb, kT, 1))
        ps = ps_misc.tile([128, 512], f32, tag="misc", name=f"ps{which}{s}{hp}")
        for cc in range(CCH):
            nc.tensor.matmul(ps[:], w_sb[:, hp, cc, :], x_sb[:, cc, ts],
                             start=(cc == 0), stop=(cc == CCH - 1))
        nc.vector.tensor_scalar_add(dst[hp][:, ts], ps[:],
                                    bqk_sb[:, bi, hp:hp + 1])

    def emit_y_group(j, t4, n):
        tt = 4 * j + t4
        psy = ps_misc.tile([128, 512], f32, tag="misc", name=f"psy{tt}{n}")
        for hp in range(FT):
            nc.tensor.matmul(
                psy[:], oT[hp][:, t4 * 128 + j * 512:t4 * 128 + j * 512 + 128],
                wp_sb[:, hp, n * 512:(n + 1) * 512],
                start=(hp == 0), stop=(hp == FT - 1))
        y_sb = tmp_pool.tile([128, 512], f32, tag="ysb")
        nc.vector.tensor_copy(y_sb[:], psy[:])
        for h in range(2):
            nc.sync.dma_start(
                out=y[tt * 128:(tt + 1) * 128,
                      n * 512 + h * 256:n * 512 + h * 256 + 256],
                in_=y_sb[:, h * 256:h * 256 + 256])

    def proj_fillers(s, first=False):
        fs = []
        if first:
            # q0/k0 first so stage-0 attention can start while wv streams in
            fs.append(lambda: emit_qk_group(s, 0, "q"))
            fs.append(lambda: emit_qk_group(s, 0, "k"))
        for t4 in range(4):
            fs.append(lambda t4=t4: emit_v_group(s, t4))
        for hp in range(0 if not first else 1, FT):
            fs.append(lambda hp=hp: emit_qk_group(s, hp, "q"))
            fs.append(lambda hp=hp: emit_qk_group(s, hp, "k"))
        return fs

    def y_fillers(j):
        return [
            (lambda t4=t4, n=n: emit_y_group(j, t4, n))
            for t4 in range(4) for n in range(2)
        ]

    # ---- HAM warm-up: ~3.5us of throwaway matmuls on the (early-arriving)
    # cmask tile so the PE clock gate is already at 8/8 when the first real
    # projection data lands; otherwise the first ~14 matmuls run at 1.2GHz.
    warm_ps = ps_misc.tile([128, 256], f32, tag="misc", name="warm")
    for _ in range(14):
        nc.tensor.matmul(warm_ps[:], cmask_sb[:, 0:128], cmask_sb[:, 0:256],
                         start=True, stop=True)

    # ---- prologue: projections for token block 0 ----
    for f in proj_fillers(0, first=True):
        f()

    # ---- stages: attention(j=s) with proj(s+1) and y(j<s-1) interleaved ----
    # Filler placement follows the per-stage ACT load: exp work grows with j
    # (nk = 4j+4 tiles) so late stages get the y-projection groups, keeping
    # the PE fed while ScalarE churns through the j=3 exp stream.
    stage_fillers = {
        0: lambda: proj_fillers(1),
        1: lambda: proj_fillers(2),
        2: lambda: proj_fillers(3),
        3: lambda: y_fillers(0) + y_fillers(1) + y_fillers(2),
    }


    for s in range(NQ):
        j = s
        nk = 4 * j + 4
        fillers = stage_fillers[s]()
        total_iters = FT * nk
        it = fi = 0

        for hp in range(FT):
            o_ps = [ps_o.tile([128, 512], f32, tag="o", name=f"o{h2}")
                    for h2 in range(2)]

            def emit_s(i):
                # straddle tiles (r>0) only touch q >= 128*r within the
                # q-tile; the PSUM zero-fill from the i==0 start covers the
                # untouched (causally masked) columns.
                r = i - 4 * j
                qo = 128 * r if r > 0 else 0
                s2 = ps_s.tile([128, 1024], f32, tag="s", name=f"s2_{i%2}")
                for h2 in range(2):
                    lo = h2 * 64
                    nc.tensor.matmul(s2[:, 512 * h2 + qo:512 * h2 + 512],
                                     kT[hp][lo:lo + 64, i * 128:(i + 1) * 128],
                                     qT[hp][lo:lo + 64,
                                            j * 512 + qo:(j + 1) * 512],
                                     start=True, stop=True)
                # one exp for both heads across the 2-bank PSUM tile
                s2_r = s2.rearrange("p (two q) -> p two q", two=2)
                pt = pt_pool.tile([128, 1024], bf16, tag="pt")
                pt_r = pt.rearrange("p (two q) -> p two q", two=2)
                nc.scalar.activation(pt_r[:, :, qo:512], s2_r[:, :, qo:512],
                                     Exp, scale=SCALE)
                if r >= 0:
                    # causal edge: first 128 valid columns get the triangle
                    nc.vector.tensor_mul(pt_r[:, :, qo:qo + 128],
                                         pt_r[:, :, qo:qo + 128], mask2[:])
                return pt, qo

            def emit_pv(i, pt, qo):
                for h2 in range(2):
                    nc.tensor.matmul(o_ps[h2][0:D + 1, qo:512],
                                     v_tiles[i][:, 2 * hp + h2, :],
                                     pt[:, 512 * h2 + qo:512 * h2 + 512],
                                     start=(i == 0), stop=(i == nk - 1))

            # Software-pipeline S ahead of PV by one tile: exp(i) then runs
            # while the PE does PV(i-1) + fillers, so PV(i) never waits on
            # the ScalarE exp stream.
            pending = None
            for i in range(nk):
                cur = (i, *emit_s(i))
                if pending is not None:
                    emit_pv(*pending)
                pending = cur
                it += 1
                while fi * total_iters < len(fillers) * it:
                    fillers[fi]()
                    fi += 1
            emit_pv(*pending)
            # ---- normalize: divide rows 0..63 by the sums row (64) ----
            sums = norm_pool.tile([65, 1024], f32, tag="sums")
            nc.vector.tensor_copy(sums[64:65, 0:512], o_ps[0][64:65, 0:512])
            nc.vector.tensor_copy(sums[64:65, 512:1024],
                                  o_ps[1][64:65, 0:512])
            # (a K=1 PE-matmul broadcast from partition 64 passes CoreSim but
            # produces garbage on hardware — keep the proven DMA+gpsimd path)
            # The odd head goes first throughout: its path is longer (extra
            # SBUF->SBUF shift DMA), and reciprocal halves are split so the
            # gpsimd broadcast of one half overlaps the reciprocal of the
            # other.
            sums_lo = norm_pool.tile([1, 1024], f32, tag="sums_lo")
            for h in range(2):
                nc.sync.dma_start(out=sums_lo[0:1, 512 * h:512 * h + 512],
                                  in_=sums[64:65, 512 * h:512 * h + 512])
            recs = norm_pool.tile([1, 1024], f32, tag="recs")
            nc.vector.reciprocal_approx_fast(recs[0:1, 512:1024],
                                             sums_lo[0:1, 512:1024])
            bc_o = norm_pool.tile([64, 512], f32, tag="bc_o")
            nc.gpsimd.partition_broadcast(bc_o[:], recs[0:1, 512:1024],
                                          channels=64)
            nc.vector.reciprocal_approx_fast(recs[0:1, 0:512],
                                             sums_lo[0:1, 0:512])
            tmp = tmp_pool.tile([64, 512], bf16, tag="tmp")
            nc.vector.tensor_mul(tmp[:], o_ps[1][0:64, :], bc_o[:])
            bc_e = norm_pool.tile([64, 512], f32, tag="bc_e")
            nc.gpsimd.partition_broadcast(bc_e[:], recs[0:1, 0:512],
                                          channels=64)
            for h in range(4):
                cs = slice(j * 512 + h * 128, j * 512 + h * 128 + 128)
                nc.sync.dma_start(out=oT[hp][64:128, cs],
                                  in_=tmp[:, h * 128:h * 128 + 128])
            nc.vector.tensor_mul(oT[hp][0:64, j * 512:(j + 1) * 512],
                                 o_ps[0][0:64, :], bc_e[:])
            if fi < len(fillers):
                fillers[fi]()
                fi += 1
            if dbg is not None and hp == 0 and j == 0:
                nc.sync.dma_start(out=dbg["d_recs0"][:], in_=recs[0:1, :])
        while fi < len(fillers):
            fillers[fi]()
            fi += 1

    # ---- epilogue: final q-tile's output projection ----
    for f in y_fillers(NQ - 1):
        f()

    if dbg is not None:
        nc.sync.dma_start(out=dbg["d_qT0"][:], in_=qT[0][:])
        nc.sync.dma_start(out=dbg["d_kT0"][:], in_=kT[0][:])
        nc.sync.dma_start(out=dbg["d_v0"][:], in_=v_tiles[0][:])
        for hp in range(FT):
            nc.sync.dma_start(out=dbg[f"d_oT{hp}"][:], in_=oT[hp][:])

    for p in reversed(pools):
        p.release()


def make_in_maps(x, Wq, bq, Wk, bk, Wv, bv, Wp, bp):
    x = np.asarray(x, dtype=np.float32)
    Wq, Wk, Wv, Wp = (np.asarray(a, dtype=np.float32) for a in (Wq, Wk, Wv, Wp))
    bq, bk, bv, bp = (np.asarray(a, dtype=np.float32) for a in (bq, bk, bv, bp))
    b16 = ml_dtypes.bfloat16
    in_maps = []
    for g in range(N_CORES):
        b, half = g // 2, g % 2
        fs = slice(half * F, (half + 1) * F)
        # [C, 128f] -> [p, hp, k, ff] with c = k*128 + p, f = hp*128 + ff
        def shuf(w):
            return np.ascontiguousarray(
                w[:, fs].reshape(CCH, 128, FT, 128).transpose(1, 2, 0, 3)
                .astype(b16))
        # bqk[p, b, hp] = bias_b[hp*128 + p]
        bqk_h = np.ascontiguousarray(
            np.stack([bq[fs], bk[fs]]).reshape(2, FT, 128)
            .transpose(2, 0, 1))
        in_maps.append({
            # xT[p, k, t] = x[b][t, k*128+p]
            "xT": np.ascontiguousarray(
                x[b].T.reshape(CCH, 128, T).transpose(1, 0, 2).astype(b16)),
            "wq": shuf(Wq),
            "wk": shuf(Wk),
            # wv[p, k, f] = Wv[k*128+p, fs][f]
            "wv": np.ascontiguousarray(
                Wv[:, fs].reshape(CCH, 128, F).transpose(1, 0, 2).astype(b16)),
            # wp[p, k, c] = Wp[fs][k*128+p, c]
            "wp": np.ascontiguousarray(
                Wp[fs, :].reshape(FT, 128, C).transpose(1, 0, 2).astype(b16)),
            "bqk": bqk_h,
            "cmask": _cmask(),
        })
    return in_maps


def _cmask():
    if "cmask" not in _cache:
        q = np.arange(128, dtype=np.float64)[None, :]
        kk = np.arange(128, dtype=np.float64)[:, None]
        tri = (q >= kk).astype(np.float32)
        c = np.concatenate([tri, tri, np.ones((128, 8), np.float32)], axis=1)
        _cache["cmask"] = np.ascontiguousarray(c.astype(ml_dtypes.bfloat16))
    return _cache["cmask"]


def gather(results, bv, Wv, Wp, bp):
    bias_total = (np.asarray(bv, np.float32) @ np.asarray(Wp, np.float32)
                  + np.asarray(bp, np.float32))
    y = np.empty((B, T, C), dtype=np.float32)
    for b in range(B):
        y[b] = results[2 * b]["y"] + results[2 * b + 1]["y"] + bias_total
    return y


def get_nc():
    if "nc" not in _cache:
        _cache["nc"] = _build()
    return _cache["nc"]


def kernel(x, Wq, bq, Wk, bk, Wv, bv, Wp, bp):
    nc = get_nc()
    in_maps = make_in_maps(x, Wq, bq, Wk, bk, Wv, bv, Wp, bp)
    res = run_bass_kernel_spmd(nc, in_maps, list(range(N_CORES)))
    return gather(res.results, bv, Wv, Wp, bp)

